# revision 9
# baseline (speedup 1.0000x reference)
"""Trainium2 Bass kernel for masked general attention (ragged sequences).

reference computation per batch b:
    q       = query[b] @ W_in.T                      [Lq, D]
    S       = q @ context[b].T                       [Lq, Lk]
    S_m     = where(qmask & kmask, S, -1e9)
    W       = softmax(S_m, axis=-1)
    mix     = W @ context[b]                         [Lq, D]
    out     = tanh(concat([mix, q]) @ W_out.T)       [Lq, D]
    returns (out, S_m)

Strategy (v2):
- Data-parallel over batch: 32 batches / 8 cores, SPMD, 4 "slots" per
  core chosen by an annealed partition minimizing the baked cost.
- All matmul operands fp16 (psum accumulation fp32); scores chain error
  is well inside the output tolerance.
- Per-slot path choices baked into the program:
    * scores lhs: per-batch context projection cw = W_in^T-applied
      context (cheap when many q-blocks) OR per-block query projection
      qp = W_in-applied queries (cheap when few q-blocks, many k).
    * out stage: mix-then-W_out (baseline) OR precomputed
      ctxWo1 = context @ Wo1 so out accumulates W~@ctxWo1 + q@Wf in one
      psum, skipping the mix matmul entirely.
- Per-block-index k-tile profile: block j only covers the max context
  tiles among batches still active at that q-depth; masking makes the
  shared program correct for every core.
- Softmax-weight transposes via HWDGE DMA-transpose on the ACT queue
  (PE transposes removed).
- Ragged semantics: skipped score regions get the exact -1e9 constant;
  fully-masked query rows get uniform-softmax semantics either
  naturally (full-width blocks) or via a rank-1 correction.
"""

import sys

sys.path.insert(0, "/opt/trn_rl_repo")

import math
import random

import numpy as np

import concourse.bass as bass
import concourse.tile as tile
from concourse import bacc, mybir
from concourse import bass_utils
from concourse.masks import make_identity

F32 = mybir.dt.float32
FP16 = mybir.dt.float16

B, Lq, Lk, D = 32, 1024, 1024, 1024
N_CORES = 8
BPC = B // N_CORES
MQ = 256
NBLK = Lq // MQ
NEG = -1e9
BIG = 3.0e38

_cache = {}

OV = 33.0  # modeled per-MM overhead ns


def _chunks(S):
    out = []
    while S > 512:
        out.append(512)
        S -= 512
    if S:
        out.append(S)
    return out


def _mm(cols, n):
    return cols / 2.4 + OV * n


_SC = {nk: sum(2 * _mm(8 * w, 8) for w in _chunks(128 * nk)) for nk in range(1, 9)}
_OUT_CTX = {nk: 4 * _mm(512 * (nk + 9), nk + 9) for nk in range(1, 9)}
_OUT_MIX = {nk: 8 * _mm(256 * nk, nk) + 8 * _mm(256, 1) + 4 * _mm(512 * 16, 16)
            for nk in range(1, 9)}
_QPROJ = 8 * _mm(256 * 8, 8)
_CW = {nk: sum(8 * _mm(8 * w, 8) for w in _chunks(128 * nk)) for nk in range(1, 9)}
_CTXW = {nk: 16 * _mm(512 * nk, nk) for nk in range(1, 9)}
_SKIP = 4 * _mm(512 * 9, 9)


def _slot_eval(key):
    """key: sorted tuple of (nq, nk) pairs. Returns (cost, use_ctx, use_qproj)."""
    NK = max(nk for _, nk in key)
    m = -(-max(nq for nq, _ in key) // 2)
    prof = []
    for j in range(m):
        prof.append(max(nk for nq, nk in key if nq > 2 * j))
    best = None
    for use_ctx in (False, True):
        for use_qproj in (False, True):
            c = (4 - m) * _SKIP
            c += (m * _QPROJ) if use_qproj else _CW[NK]
            if use_ctx:
                c += _CTXW[NK]
            for nk in prof:
                c += _SC[nk] + (_OUT_CTX[nk] if use_ctx else _OUT_MIX[nk])
            if best is None or c < best[0]:
                best = (c, use_ctx, use_qproj)
    return best


def _assign_slots(query_lengths, context_lengths):
    """Partition the 32 batches into 4 slots x 8 cores minimizing the baked
    cost. Returns (groups, slot_params) with groups[s] = 8 batch ids and
    slot_params[s] = (m, NK, prof, use_ctx, use_qproj)."""
    nqt = [int(-(-int(q) // 128)) for q in query_lengths]
    nkt = [int(-(-int(c) // 128)) for c in context_lengths]
    cache = {}

    def slot_cost(g):
        key = tuple(sorted((nqt[i], nkt[i]) for i in g))
        r = cache.get(key)
        if r is None:
            r = _slot_eval(key)
            cache[key] = r
        return r[0]

    best_overall = None
    for seed in range(6):
        rng = random.Random(seed)
        if seed % 2 == 0:
            order = sorted(range(B), key=lambda i: nqt[i] * nkt[i])
        else:
            order = list(range(B))
            rng.shuffle(order)
        groups = [order[j * 8:(j + 1) * 8] for j in range(4)]
        cur = sum(slot_cost(g) for g in groups)
        best = cur
        bestg = [list(g) for g in groups]
        N = 60000
        for it in range(N):
            a = rng.randrange(4)
            b = rng.randrange(4)
            if a == b:
                continue
            i = rng.randrange(8)
            j = rng.randrange(8)
            d0 = slot_cost(groups[a]) + slot_cost(groups[b])
            groups[a][i], groups[b][j] = groups[b][j], groups[a][i]
            d1 = slot_cost(groups[a]) + slot_cost(groups[b])
            c = cur - d0 + d1
            T = 3000.0 * (1 - it / N) + 0.5
            if c <= cur or rng.random() < math.exp((cur - c) / T):
                cur = c
                if c < best:
                    best = c
                    bestg = [list(g) for g in groups]
            else:
                groups[a][i], groups[b][j] = groups[b][j], groups[a][i]
        if best_overall is None or best < best_overall[0]:
            best_overall = (best, bestg)

    groups = best_overall[1]
    sp = []
    for g in groups:
        key = tuple(sorted((nqt[i], nkt[i]) for i in g))
        c, use_ctx, use_qproj = _slot_eval(key)
        NK = max(nkt[i] for i in g)
        m = -(-max(nqt[i] for i in g) // 2)
        prof = tuple(max(nkt[i] for i in g if nqt[i] > 2 * j) for j in range(m))
        sp.append((m, NK, prof, use_ctx, use_qproj, c))
    # order slots: qproj slots last, others largest-first (dense PE start,
    # long crossing windows for the next slot's context prefetch)
    idx = sorted(range(4), key=lambda s: (sp[s][4], -sp[s][5]))
    groups = [groups[s] for s in idx]
    params = tuple((sp[s][0], sp[s][1], sp[s][2], sp[s][3], sp[s][4])
                   for s in idx)
    return groups, params


def _build_program(params):
    """params: tuple of (m, NK, prof, use_ctx, use_qproj) per slot."""
    nc = bacc.Bacc("TRN2", target_bir_lowering=False, debug=False,
                   num_devices=N_CORES)

    any_cw = any(not p[4] for p in params)
    any_qp = any(p[4] for p in params)
    any_ctx = any(p[3] for p in params)
    any_mix = any(not p[3] for p in params)

    qT16_d = nc.dram_tensor("qT16", [BPC, D, Lq], FP16, kind="ExternalInput").ap()
    cT_d = nc.dram_tensor("cT", [BPC, D, Lk], FP16, kind="ExternalInput").ap()
    cn_d = nc.dram_tensor("cn", [BPC, Lk, D], FP16, kind="ExternalInput").ap()
    win_d = nc.dram_tensor("win", [D, D], FP16, kind="ExternalInput").ap()
    winT_d = nc.dram_tensor("winT", [D, D], FP16, kind="ExternalInput").ap()
    wo1_d = nc.dram_tensor("wo1", [D, D], FP16, kind="ExternalInput").ap()
    wf_d = nc.dram_tensor("wf", [D, D], FP16, kind="ExternalInput").ap()
    kmin_d = nc.dram_tensor("kmin", [BPC, 128, Lk], F32, kind="ExternalInput").ap()
    qmin_d = nc.dram_tensor("qmin", [BPC, 128, Lq // 128], F32, kind="ExternalInput").ap()
    q01_d = nc.dram_tensor("q01", [BPC, 128, Lq // 128], F32, kind="ExternalInput").ap()
    m01_d = nc.dram_tensor("m01", [BPC, Lq], FP16, kind="ExternalInput").ap()
    mean_d = nc.dram_tensor("mean", [BPC, D], FP16, kind="ExternalInput").ap()
    cb_d = nc.dram_tensor("cb", [BPC, D], FP16, kind="ExternalInput").ap()

    out_d = nc.dram_tensor("out", [BPC, Lq, D], F32, kind="ExternalOutput").ap()
    sc_d = nc.dram_tensor("sc", [BPC, Lq, Lk], F32, kind="ExternalOutput").ap()

    with tile.TileContext(nc) as tc:
        with (
            tc.tile_pool(name="static", bufs=1) as st,
            tc.tile_pool(name="ctx", bufs=1) as ctx_pool,
            tc.tile_pool(name="qry", bufs=2) as qry_pool,
            tc.tile_pool(name="qp", bufs=2) as qp_pool,
            tc.tile_pool(name="q16s", bufs=2) as q16s_pool,
            tc.tile_pool(name="ew", bufs=2) as ew_pool,
            tc.tile_pool(name="wt", bufs=2) as wt_pool,
            tc.tile_pool(name="sm", bufs=2) as sm_pool,
            tc.tile_pool(name="ot", bufs=3) as ot_pool,
            tc.tile_pool(name="stats", bufs=4) as stats_pool,
            tc.tile_pool(name="psS", bufs=2, space="PSUM") as psS,
            tc.tile_pool(name="psO", bufs=2, space="PSUM") as psO,
            tc.tile_pool(name="psP", bufs=2, space="PSUM") as psP,
            tc.tile_pool(name="psT", bufs=2, space="PSUM") as psT,
        ):

            def qry16_dma(s, blk_i, pool=None, tag="qry"):
                q0 = blk_i * MQ
                pool = pool or qry_pool
                t = pool.tile([128, 8 * MQ], FP16, tag=tag)
                nc.sync.dma_start(
                    t[:].rearrange("p (c x) -> p c x", c=8),
                    qT16_d[s].rearrange("(c p) q -> p c q", p=128)[:, :, q0:q0 + MQ])
                return t

            def load_cT(s):
                m_, NK, prof, use_ctx, use_qp = params[s]
                S = 128 * NK
                cT_sb = ctx_pool.tile([128, 8 * Lk], FP16, tag="cT")
                nc.sync.dma_start(
                    cT_sb[:].rearrange("p (c x) -> p c x", c=8)[:, :, :S],
                    cT_d[s].rearrange("(c p) k -> p c k", p=128)[:, :, :S])
                return dict(cT=cT_sb)

            def load_masks(s, ctx):
                m_, NK, prof, use_ctx, use_qp = params[s]
                S = 128 * NK
                kmin_sb = ctx_pool.tile([128, Lk], F32, tag="kmin")
                nc.sync.dma_start(kmin_sb[:, :S], kmin_d[s, :, :S])
                qmin_sb = ctx_pool.tile([128, Lq // 128], F32, tag="qmin")
                nc.sync.dma_start(qmin_sb[:], qmin_d[s])
                q01_sb = ctx_pool.tile([128, Lq // 128], F32, tag="q01")
                nc.sync.dma_start(q01_sb[:], q01_d[s])
                ctx.update(kmin=kmin_sb, qmin=qmin_sb, q01=q01_sb)

            def cw_build(s, ctx):
                """cw[d, k] = sum_e W_in[e, d] * contextT[e, k] (fp16)."""
                m_, NK, prof, use_ctx, use_qp = params[s]
                cw_sb = ctx_pool.tile([128, 8 * Lk], FP16, tag="cw")
                chs = _chunks(128 * NK)
                for dt in range(8):
                    off = 0
                    for w in chs:
                        ps = psP.tile([128, 512], F32, tag="psP")
                        for et in range(8):
                            nc.tensor.matmul(
                                ps[:, :w],
                                win_sb[:, et * D + dt * 128:et * D + (dt + 1) * 128],
                                ctx["cT"][:, et * Lk + off:et * Lk + off + w],
                                start=(et == 0), stop=(et == 7))
                        nc.vector.tensor_copy(
                            cw_sb[:, dt * Lk + off:dt * Lk + off + w],
                            ps[:, :w])
                        off += w
                ctx["cw"] = cw_sb

            def ctxwo1_build(s, ctx):
                """kd[k, d'] = sum_d context[k, d] * Wo1[d, d'] (fp16),
                k-tile kt in columns kt*D..(kt+1)*D. Emitted late (after the
                previous slot's last out) because the kd tag frees there."""
                m_, NK, prof, use_ctx, use_qp = params[s]
                kd_sb = ctx_pool.tile([128, 8 * D], FP16, tag="kd")
                for kt in range(NK):
                    for n in range(2):
                        ps = psP.tile([128, 512], F32, tag="psP")
                        for dt in range(8):
                            nc.tensor.matmul(
                                ps[:],
                                ctx["cT"][:, dt * Lk + kt * 128:dt * Lk + (kt + 1) * 128],
                                wo1_sb[:, dt * D + n * 512:dt * D + (n + 1) * 512],
                                start=(dt == 0), stop=(dt == 7))
                        nc.vector.tensor_copy(
                            kd_sb[:, kt * D + n * 512:kt * D + (n + 1) * 512],
                            ps[:])
                ctx["kd"] = kd_sb

            def load_ctx_late(s, ctx):
                m_, NK, prof, use_ctx, use_qp = params[s]
                if use_ctx:
                    ctxwo1_build(s, ctx)
                else:
                    kd_sb = ctx_pool.tile([128, 8 * D], FP16, tag="kd")
                    nc.sync.dma_start(
                        kd_sb[:].rearrange("p (c d) -> p c d", c=8)[:, :NK],
                        cn_d[s].rearrange("(c p) d -> p c d", p=128)[:, :NK])
                    ctx["kd"] = kd_sb
                    mean_sb = ctx_pool.tile([1, D], FP16, tag="mean")
                    nc.sync.dma_start(mean_sb[:], mean_d[s:s + 1, :])
                    ctx["mean"] = mean_sb
                m01_sb = ctx_pool.tile([1, Lq], FP16, tag="m01")
                nc.sync.dma_start(m01_sb[:], m01_d[s:s + 1, :])
                cb_sb = ctx_pool.tile([1, D], FP16, tag="cb")
                nc.sync.dma_start(cb_sb[:], cb_d[s:s + 1, :])
                ctx.update(m01=m01_sb, cb=cb_sb)

            def qproj(s, blk_i, q16_sb):
                """qp[e, q] = sum_d W_in[e, d] * queryT[d, q] (fp16)."""
                qp_sb = qp_pool.tile([128, 8 * MQ], FP16, tag="qp")
                for et in range(8):
                    ps = psP.tile([128, 512], F32, tag="psP")
                    for dt in range(8):
                        nc.tensor.matmul(
                            ps[:, :MQ],
                            winT_sb[:, dt * D + et * 128:dt * D + (et + 1) * 128],
                            q16_sb[:, dt * MQ:(dt + 1) * MQ],
                            start=(dt == 0), stop=(dt == 7))
                    nc.vector.tensor_copy(qp_sb[:, et * MQ:(et + 1) * MQ],
                                          ps[:, :MQ])
                return qp_sb

            const_sb = st.tile([128, Lk], F32, tag="const")
            nc.vector.memset(const_sb[:], NEG)
            ones_sb = st.tile([1, 128], FP16, tag="ones")
            nc.vector.memset(ones_sb[:], 1.0)

            def scores_softmax(s, blk_i, qsrc_sb, ctx):
                """Masked scores -> DRAM; softmax weights -> ew tile (fp16)."""
                m_, NK, prof, use_ctx, use_qp = params[s]
                nk = prof[blk_i]
                S = 128 * nk
                chs = _chunks(S)
                NCH = len(chs)
                rhs = ctx["cT"] if use_qp else ctx["cw"]
                q0 = blk_i * MQ
                ew_sb = ew_pool.tile([128, 2 * Lk], FP16, tag="ew")
                for h in range(2):
                    jt = blk_i * 2 + h
                    rows = slice(q0 + h * 128, q0 + (h + 1) * 128)
                    stt = stats_pool.tile([128, 8], F32, tag="stats")
                    ps_n = [psS.tile([128, 512], F32, tag="psS", name="ps_n") for _ in chs]
                    for dt in range(8):
                        off = 0
                        for n, w in enumerate(chs):
                            nc.tensor.matmul(
                                ps_n[n][:, :w],
                                qsrc_sb[:, dt * MQ + h * 128:dt * MQ + (h + 1) * 128],
                                rhs[:, dt * Lk + off:dt * Lk + off + w],
                                start=(dt == 0), stop=(dt == 7))
                            off += w
                    sm_n = []
                    off = 0
                    for n, w in enumerate(chs):
                        sm = sm_pool.tile([128, 512], F32, tag="sm")
                        sm_n.append((sm, off, w))
                        nc.vector.tensor_tensor(
                            sm[:, :w], ps_n[n][:, :w], ctx["kmin"][:, off:off + w],
                            op=mybir.AluOpType.min)
                        nc.vector.tensor_scalar_min(
                            sm[:, :w], sm[:, :w], ctx["qmin"][:, jt:jt + 1])
                        nc.sync.dma_start(sc_d[s, rows, off:off + w], sm[:, :w])
                        nc.vector.reduce_max(
                            stt[:, n:n + 1], sm[:, :w],
                            axis=mybir.AxisListType.X, negate=True)
                        off += w
                    if S < Lk:
                        nc.sync.dma_start(sc_d[s, rows, S:], const_sb[:, :Lk - S])
                    if NCH == 1:
                        negm = stt[:, 0:1]
                    else:
                        nc.vector.tensor_tensor(
                            stt[:, 2:3], stt[:, 0:1], stt[:, 1:2],
                            op=mybir.AluOpType.min)
                        negm = stt[:, 2:3]
                    for n, (sm, off, w) in enumerate(sm_n):
                        nc.scalar.activation(
                            ew_sb[:, h * Lk + off:h * Lk + off + w],
                            sm[:, :w],
                            mybir.ActivationFunctionType.Exp,
                            bias=negm, scale=1.0,
                            accum_out=stt[:, 3 + n:4 + n])
                    if NCH == 1:
                        ssum = stt[:, 3:4]
                    else:
                        nc.vector.tensor_tensor(
                            stt[:, 5:6], stt[:, 3:4], stt[:, 4:5],
                            op=mybir.AluOpType.add)
                        ssum = stt[:, 5:6]
                    nc.vector.reciprocal(stt[:, 6:7], ssum)
                    if nk == 8:
                        scale = stt[:, 6:7]
                    else:
                        nc.vector.tensor_tensor(
                            stt[:, 7:8], stt[:, 6:7], ctx["q01"][:, jt:jt + 1],
                            op=mybir.AluOpType.mult)
                        scale = stt[:, 7:8]
                    nc.vector.tensor_scalar_mul(
                        ew_sb[:, h * Lk:h * Lk + S],
                        ew_sb[:, h * Lk:h * Lk + S],
                        scale)
                return ew_sb

            def transposes(s, blk_i, ew_sb):
                """W^T via PE transpose-mode (pipelines ~190ns/tile)."""
                m_, NK, prof, use_ctx, use_qp = params[s]
                nk = prof[blk_i]
                wt_sb = wt_pool.tile([128, 8 * MQ], FP16, tag="wt")
                for kt in range(nk):
                    pt = psT.tile([128, MQ], FP16, tag="psT")
                    for h in range(2):
                        nc.tensor.transpose(
                            pt[:, h * 128:(h + 1) * 128],
                            ew_sb[:, h * Lk + kt * 128:h * Lk + (kt + 1) * 128],
                            ident[:])
                    nc.vector.tensor_copy(wt_sb[:, kt * MQ:(kt + 1) * MQ], pt[:])
                return wt_sb

            def out_ctx(s, blk_i, q16_sb, wt_sb, ctx):
                """out = tanh(W~@ctxWo1 + q@Wf [+ rank1 cb]) in one psum."""
                m_, NK, prof, use_ctx, use_qp = params[s]
                nk = prof[blk_i]
                q0 = blk_i * MQ
                for h in range(2):
                    rows = slice(q0 + h * 128, q0 + (h + 1) * 128)
                    po = [psO.tile([128, 512], F32, tag="psO", name="po") for _ in range(2)]
                    for kt in range(nk):
                        for n in range(2):
                            nc.tensor.matmul(
                                po[n][:],
                                wt_sb[:, kt * MQ + h * 128:kt * MQ + (h + 1) * 128],
                                ctx["kd"][:, kt * D + n * 512:kt * D + (n + 1) * 512],
                                start=(kt == 0), stop=False)
                    for dt in range(8):
                        for n in range(2):
                            nc.tensor.matmul(
                                po[n][:],
                                q16_sb[:, dt * MQ + h * 128:dt * MQ + (h + 1) * 128],
                                wf_sb[:, dt * D + n * 512:dt * D + (n + 1) * 512],
                                start=False, stop=(dt == 7 and nk == 8))
                    if nk < 8:
                        for n in range(2):
                            nc.tensor.matmul(
                                po[n][:],
                                ctx["m01"][0:1, q0 + h * 128:q0 + (h + 1) * 128],
                                ctx["cb"][0:1, n * 512:(n + 1) * 512],
                                start=False, stop=True)
                    for n in range(2):
                        ot = ot_pool.tile([128, 512], F32, tag="ot")
                        nc.scalar.activation(
                            ot[:], po[n][:], mybir.ActivationFunctionType.Tanh)
                        nc.sync.dma_start(out_d[s, rows, n * 512:(n + 1) * 512],
                                          ot[:])

            def out_mix(s, blk_i, q16_sb, wt_sb, ctx):
                """Baseline path: mixT = kd-chunks @ wt, out via Wo1 + Wf."""
                m_, NK, prof, use_ctx, use_qp = params[s]
                nk = prof[blk_i]
                q0 = blk_i * MQ
                mixT_sb = wt_pool.tile([128, 8 * MQ], FP16, tag="mixT")
                for dt in range(8):
                    pm = psO.tile([128, 512], F32, tag="psO")
                    for kt in range(nk):
                        nc.tensor.matmul(
                            pm[:, :MQ],
                            ctx["kd"][:, kt * D + dt * 128:kt * D + (dt + 1) * 128],
                            wt_sb[:, kt * MQ:(kt + 1) * MQ],
                            start=(kt == 0), stop=(kt == nk - 1 and nk == 8))
                    if nk < 8:
                        nc.tensor.matmul(
                            pm[:, :MQ],
                            ctx["mean"][0:1, dt * 128:(dt + 1) * 128],
                            ctx["m01"][0:1, q0:q0 + MQ],
                            start=False, stop=True)
                    nc.vector.tensor_copy(mixT_sb[:, dt * MQ:(dt + 1) * MQ],
                                          pm[:, :MQ])
                for h in range(2):
                    rows = slice(q0 + h * 128, q0 + (h + 1) * 128)
                    po = [psO.tile([128, 512], F32, tag="psO", name="po") for _ in range(2)]
                    for dt in range(8):
                        for n in range(2):
                            nc.tensor.matmul(
                                po[n][:],
                                mixT_sb[:, dt * MQ + h * 128:dt * MQ + (h + 1) * 128],
                                wo1_sb[:, dt * D + n * 512:dt * D + (n + 1) * 512],
                                start=(dt == 0), stop=False)
                    for dt in range(8):
                        for n in range(2):
                            nc.tensor.matmul(
                                po[n][:],
                                q16_sb[:, dt * MQ + h * 128:dt * MQ + (h + 1) * 128],
                                wf_sb[:, dt * D + n * 512:dt * D + (n + 1) * 512],
                                start=False, stop=(dt == 7))
                    for n in range(2):
                        ot = ot_pool.tile([128, 512], F32, tag="ot")
                        nc.scalar.activation(
                            ot[:], po[n][:], mybir.ActivationFunctionType.Tanh)
                        nc.sync.dma_start(out_d[s, rows, n * 512:(n + 1) * 512],
                                          ot[:])

            def skipped_block(s, blk_i, ctx):
                """q-block past every query length in the slot: scores are
                all -1e9; out = tanh(query@Wf + cb)."""
                q0 = blk_i * MQ
                q16_sb = qry16_dma(s, blk_i, pool=q16s_pool, tag="q16s")
                for h in range(2):
                    rows = slice(q0 + h * 128, q0 + (h + 1) * 128)
                    nc.sync.dma_start(sc_d[s, rows, :], const_sb[:])
                    po = [psO.tile([128, 512], F32, tag="psO", name="po") for _ in range(2)]
                    for dt in range(8):
                        for n in range(2):
                            nc.tensor.matmul(
                                po[n][:],
                                q16_sb[:, dt * MQ + h * 128:dt * MQ + (h + 1) * 128],
                                wf_sb[:, dt * D + n * 512:dt * D + (n + 1) * 512],
                                start=(dt == 0), stop=False)
                    for n in range(2):
                        nc.tensor.matmul(
                            po[n][:], ones_sb[0:1, :],
                            ctx["cb"][0:1, n * 512:(n + 1) * 512],
                            start=False, stop=True)
                    for n in range(2):
                        ot = ot_pool.tile([128, 512], F32, tag="ot")
                        nc.scalar.activation(
                            ot[:], po[n][:], mybir.ActivationFunctionType.Tanh)
                        nc.sync.dma_start(out_d[s, rows, n * 512:(n + 1) * 512],
                                          ot[:])

            # ---- prologue
            ctx0 = load_cT(0)
            if any_cw:
                win_sb = st.tile([128, 8 * D], FP16, tag="win")
                nc.sync.dma_start(
                    win_sb[:].rearrange("p (e d) -> p e d", e=8),
                    win_d.rearrange("(e p) d -> p e d", p=128))
            if any_qp:
                winT_sb = st.tile([128, 8 * D], FP16, tag="winT")
                nc.sync.dma_start(
                    winT_sb[:].rearrange("p (c d) -> p c d", c=8),
                    winT_d.rearrange("(c p) e -> p c e", p=128))
            qry0 = qry16_dma(0, 0)
            load_masks(0, ctx0)

            if any_ctx or any_mix:
                wo1_sb = st.tile([128, 8 * D], FP16, tag="wo1")
                nc.sync.dma_start(
                    wo1_sb[:].rearrange("p (c d) -> p c d", c=8),
                    wo1_d.rearrange("(c p) d -> p c d", p=128))
            wf_sb = st.tile([128, 8 * D], FP16, tag="wf")
            nc.sync.dma_start(
                wf_sb[:].rearrange("p (c d) -> p c d", c=8),
                wf_d.rearrange("(c p) d -> p c d", p=128))
            ident = st.tile([128, 128], FP16, tag="ident")
            make_identity(nc, ident[:])

            def emit_skips(s, ctx):
                for si in range(params[s][0], NBLK):
                    skipped_block(s, si, ctx)

            if not params[0][4]:
                cw_build(0, ctx0)
            load_ctx_late(0, ctx0)
            emit_skips(0, ctx0)

            # flattened computed-block sequence with one-block lookahead
            seq = [(s, j) for s in range(BPC) for j in range(params[s][0])]
            cur_ctx = {0: ctx0}

            def make_scores(s, j, ctx):
                q16 = qry16_dma(s, j)
                if params[s][4]:
                    qsrc = qproj(s, j, q16)
                else:
                    qsrc = q16
                ew = scores_softmax(s, j, qsrc, ctx)
                return (s, j, q16, ew)

            pend = make_scores(0, 0, ctx0)
            for idx in range(len(seq)):
                s, j = seq[idx]
                _, _, q16_sb, ew_sb = pend
                ctx = cur_ctx[s]
                if not params[s][4]:
                    for la in (2, 3):
                        if idx + la < len(seq):
                            fs = seq[idx + la][0]
                            if fs != s and fs not in cur_ctx:
                                cur_ctx[fs] = load_cT(fs)
                nxt = seq[idx + 1] if idx + 1 < len(seq) else None
                if nxt is not None:
                    ns, nj = nxt
                    if ns != s:
                        if ns not in cur_ctx:
                            cur_ctx[ns] = load_cT(ns)
                        load_masks(ns, cur_ctx[ns])
                        if not params[ns][4]:
                            cw_build(ns, cur_ctx[ns])
                    pend = make_scores(ns, nj, cur_ctx[ns])
                wt_sb = transposes(s, j, ew_sb)
                if params[s][3]:
                    out_ctx(s, j, q16_sb, wt_sb, ctx)
                else:
                    out_mix(s, j, q16_sb, wt_sb, ctx)
                if nxt is not None and nxt[0] != s:
                    load_ctx_late(nxt[0], cur_ctx[nxt[0]])
                    emit_skips(nxt[0], cur_ctx[nxt[0]])

    nc.compile()
    return nc


def kernel(query, context, query_lengths, context_lengths, W_in, W_out):
    groups, params = _assign_slots(np.asarray(query_lengths),
                                   np.asarray(context_lengths))
    if _cache.get("params") != params:
        _cache["nc"] = _build_program(params)
        _cache["params"] = params
    nc = _cache["nc"]

    query = np.asarray(query, dtype=np.float32)
    context = np.asarray(context, dtype=np.float32)
    ql = np.asarray(query_lengths).astype(np.int64)
    cl = np.asarray(context_lengths).astype(np.int64)

    qT16 = np.ascontiguousarray(query.transpose(0, 2, 1)).astype(np.float16)
    cT16 = np.ascontiguousarray(context.transpose(0, 2, 1)).astype(np.float16)
    cn16 = context.astype(np.float16)
    win = np.ascontiguousarray(W_in, dtype=np.float32).astype(np.float16)
    winT = np.ascontiguousarray(W_in.T, dtype=np.float32).astype(np.float16)
    woT = np.ascontiguousarray(W_out.T, dtype=np.float64)
    wo1 = woT[:D].astype(np.float16)
    wf = (np.asarray(W_in, dtype=np.float64).T @ woT[D:]).astype(np.float16)
    mean_c = context.astype(np.float64).mean(axis=1)
    cb = (mean_c @ woT[:D]).astype(np.float16)
    mean16 = mean_c.astype(np.float16)

    k_idx = np.arange(Lk)
    q_idx = np.arange(Lq)
    kvalid = k_idx[None, :] < cl[:, None]
    qvalid = q_idx[None, :] < ql[:, None]
    kmin = np.where(kvalid, np.float32(BIG), np.float32(NEG)).astype(np.float32)
    qmin = np.where(qvalid, np.float32(BIG), np.float32(NEG)).astype(np.float32)
    q01 = qvalid.astype(np.float32)
    m01 = (~qvalid).astype(np.float16)
    kmin_rep = np.ascontiguousarray(
        np.broadcast_to(kmin[:, None, :], (B, 128, Lk)))
    qmin_til = np.ascontiguousarray(
        qmin.reshape(B, Lq // 128, 128).transpose(0, 2, 1))
    q01_til = np.ascontiguousarray(
        q01.reshape(B, Lq // 128, 128).transpose(0, 2, 1))

    # core c processes batch groups[s][c] in slot s
    in_maps = []
    for c in range(N_CORES):
        sidx = [groups[s][c] for s in range(BPC)]
        in_maps.append({
            "qT16": np.ascontiguousarray(qT16[sidx]),
            "cT": np.ascontiguousarray(cT16[sidx]),
            "cn": np.ascontiguousarray(cn16[sidx]),
            "win": win, "winT": winT, "wo1": wo1, "wf": wf,
            "kmin": np.ascontiguousarray(kmin_rep[sidx]),
            "qmin": np.ascontiguousarray(qmin_til[sidx]),
            "q01": np.ascontiguousarray(q01_til[sidx]),
            "m01": np.ascontiguousarray(m01[sidx]),
            "mean": np.ascontiguousarray(mean16[sidx]),
            "cb": np.ascontiguousarray(cb[sidx]),
        })

    res = bass_utils.run_bass_kernel_spmd(nc, in_maps, core_ids=list(range(N_CORES)))
    _cache["last_result"] = res

    out = np.empty((B, Lq, D), dtype=np.float32)
    scores = np.empty((B, Lq, Lk), dtype=np.float32)
    for c in range(N_CORES):
        for s in range(BPC):
            bidx = groups[s][c]
            out[bidx] = res.results[c]["out"][s]
            scores[bidx] = res.results[c]["sc"][s]
    return out, scores


_program_cache = _cache  # test.py compatibility


# revision 10
# speedup vs baseline: 1.1807x; 1.1807x over previous
"""Trainium2 Bass kernel for masked general attention (ragged sequences).

reference computation per batch b:
    q       = query[b] @ W_in.T                      [Lq, D]
    S       = q @ context[b].T                       [Lq, Lk]
    S_m     = where(qmask & kmask, S, -1e9)
    W       = softmax(S_m, axis=-1)
    mix     = W @ context[b]                         [Lq, D]
    out     = tanh(concat([mix, q]) @ W_out.T)       [Lq, D]
    returns (out, S_m)

Strategy (v2):
- Data-parallel over batch: 32 batches / 8 cores, SPMD, 4 "slots" per
  core chosen by an annealed partition minimizing the baked cost.
- All matmul operands fp16 (psum accumulation fp32); scores chain error
  is well inside the output tolerance.
- Per-slot path choices baked into the program:
    * scores lhs: per-batch context projection cw = W_in^T-applied
      context (cheap when many q-blocks) OR per-block query projection
      qp = W_in-applied queries (cheap when few q-blocks, many k).
    * out stage: mix-then-W_out (baseline) OR precomputed
      ctxWo1 = context @ Wo1 so out accumulates W~@ctxWo1 + q@Wf in one
      psum, skipping the mix matmul entirely.
- Per-block-index k-tile profile: block j only covers the max context
  tiles among batches still active at that q-depth; masking makes the
  shared program correct for every core.
- Softmax-weight transposes via HWDGE DMA-transpose on the ACT queue
  (PE transposes removed).
- Ragged semantics: skipped score regions get the exact -1e9 constant;
  fully-masked query rows get uniform-softmax semantics either
  naturally (full-width blocks) or via a rank-1 correction.
"""

import sys

sys.path.insert(0, "/opt/trn_rl_repo")

import math
import random

import numpy as np

import concourse.bass as bass
import concourse.tile as tile
from concourse import bacc, mybir
from concourse import bass_utils
from concourse.masks import make_identity

F32 = mybir.dt.float32
FP16 = mybir.dt.float16

B, Lq, Lk, D = 32, 1024, 1024, 1024
N_CORES = 8
BPC = B // N_CORES
MQ = 256
NBLK = Lq // MQ
NEG = -1e9
BIG = 3.0e38

_cache = {}

OV = 33.0  # modeled per-MM overhead ns


def _chunks(S):
    out = []
    while S > 512:
        out.append(512)
        S -= 512
    if S:
        out.append(S)
    return out


def _mm(cols, n):
    return cols / 2.4 + OV * n


_SC = {nk: sum(2 * _mm(8 * w, 8) for w in _chunks(128 * nk)) for nk in range(1, 9)}
_OUT_CTX = {nk: 4 * _mm(512 * (nk + 9), nk + 9) for nk in range(1, 9)}
_OUT_MIX = {nk: 8 * _mm(256 * nk, nk) + 8 * _mm(256, 1) + 4 * _mm(512 * 16, 16)
            for nk in range(1, 9)}
_QPROJ = 8 * _mm(256 * 8, 8)
_CW = {nk: sum(8 * _mm(8 * w, 8) for w in _chunks(128 * nk)) for nk in range(1, 9)}
_CTXW = {nk: 16 * _mm(512 * nk, nk) for nk in range(1, 9)}
_SKIP = 4 * _mm(512 * 9, 9)


def _slot_eval(key):
    """key: sorted tuple of (nq, nk) pairs. Returns (cost, use_ctx, use_qproj)."""
    NK = max(nk for _, nk in key)
    m = -(-max(nq for nq, _ in key) // 2)
    prof = []
    for j in range(m):
        prof.append(max(nk for nq, nk in key if nq > 2 * j))
    best = None
    for use_ctx in (False, True):
        for use_qproj in (False, True):
            c = (4 - m) * _SKIP
            c += (m * _QPROJ) if use_qproj else _CW[NK]
            if use_ctx:
                c += _CTXW[NK]
            for nk in prof:
                c += _SC[nk] + (_OUT_CTX[nk] if use_ctx else _OUT_MIX[nk])
            if best is None or c < best[0]:
                best = (c, use_ctx, use_qproj)
    return best


def _assign_slots(query_lengths, context_lengths):
    """Partition the 32 batches into 4 slots x 8 cores minimizing the baked
    cost. Returns (groups, slot_params) with groups[s] = 8 batch ids and
    slot_params[s] = (m, NK, prof, use_ctx, use_qproj)."""
    nqt = [int(-(-int(q) // 128)) for q in query_lengths]
    nkt = [int(-(-int(c) // 128)) for c in context_lengths]
    cache = {}

    def slot_cost(g):
        key = tuple(sorted((nqt[i], nkt[i]) for i in g))
        r = cache.get(key)
        if r is None:
            r = _slot_eval(key)
            cache[key] = r
        return r[0]

    best_overall = None
    for seed in range(6):
        rng = random.Random(seed)
        if seed % 2 == 0:
            order = sorted(range(B), key=lambda i: nqt[i] * nkt[i])
        else:
            order = list(range(B))
            rng.shuffle(order)
        groups = [order[j * 8:(j + 1) * 8] for j in range(4)]
        cur = sum(slot_cost(g) for g in groups)
        best = cur
        bestg = [list(g) for g in groups]
        N = 60000
        for it in range(N):
            a = rng.randrange(4)
            b = rng.randrange(4)
            if a == b:
                continue
            i = rng.randrange(8)
            j = rng.randrange(8)
            d0 = slot_cost(groups[a]) + slot_cost(groups[b])
            groups[a][i], groups[b][j] = groups[b][j], groups[a][i]
            d1 = slot_cost(groups[a]) + slot_cost(groups[b])
            c = cur - d0 + d1
            T = 3000.0 * (1 - it / N) + 0.5
            if c <= cur or rng.random() < math.exp((cur - c) / T):
                cur = c
                if c < best:
                    best = c
                    bestg = [list(g) for g in groups]
            else:
                groups[a][i], groups[b][j] = groups[b][j], groups[a][i]
        if best_overall is None or best < best_overall[0]:
            best_overall = (best, bestg)

    groups = best_overall[1]
    sp = []
    for g in groups:
        key = tuple(sorted((nqt[i], nkt[i]) for i in g))
        c, use_ctx, use_qproj = _slot_eval(key)
        NK = max(nkt[i] for i in g)
        m = -(-max(nqt[i] for i in g) // 2)
        prof = tuple(max(nkt[i] for i in g if nqt[i] > 2 * j) for j in range(m))
        sp.append((m, NK, prof, use_ctx, use_qproj, c))
    # order slots: qproj slots last, others largest-first (dense PE start,
    # long crossing windows for the next slot's context prefetch)
    idx = sorted(range(4), key=lambda s: (sp[s][4], -sp[s][5]))
    groups = [groups[s] for s in idx]
    params = tuple((sp[s][0], sp[s][1], sp[s][2], sp[s][3], sp[s][4])
                   for s in idx)
    return groups, params


def _build_program(params):
    """params: tuple of (m, NK, prof, use_ctx, use_qproj) per slot."""
    nc = bacc.Bacc("TRN2", target_bir_lowering=False, debug=False,
                   num_devices=N_CORES)

    any_cw = any(not p[4] for p in params)
    any_qp = any(p[4] for p in params)
    any_ctx = any(p[3] for p in params)
    any_mix = any(not p[3] for p in params)

    qT16_d = nc.dram_tensor("qT16", [BPC, D, Lq], FP16, kind="ExternalInput").ap()
    cT_d = nc.dram_tensor("cT", [BPC, D, Lk], FP16, kind="ExternalInput").ap()
    cn_d = nc.dram_tensor("cn", [BPC, Lk, D], FP16, kind="ExternalInput").ap()
    win_d = nc.dram_tensor("win", [D, D], FP16, kind="ExternalInput").ap()
    winT_d = nc.dram_tensor("winT", [D, D], FP16, kind="ExternalInput").ap()
    wo1_d = nc.dram_tensor("wo1", [D, D], FP16, kind="ExternalInput").ap()
    wf_d = nc.dram_tensor("wf", [D, D], FP16, kind="ExternalInput").ap()
    kmin_d = nc.dram_tensor("kmin", [BPC, 128, Lk], F32, kind="ExternalInput").ap()
    qmin_d = nc.dram_tensor("qmin", [BPC, 128, Lq // 128], F32, kind="ExternalInput").ap()
    q01_d = nc.dram_tensor("q01", [BPC, 128, Lq // 128], F32, kind="ExternalInput").ap()
    m01_d = nc.dram_tensor("m01", [BPC, Lq], FP16, kind="ExternalInput").ap()
    mean_d = nc.dram_tensor("mean", [BPC, D], FP16, kind="ExternalInput").ap()
    cb_d = nc.dram_tensor("cb", [BPC, D], FP16, kind="ExternalInput").ap()

    out_d = nc.dram_tensor("out", [BPC, Lq, D], F32, kind="ExternalOutput").ap()
    sc_d = nc.dram_tensor("sc", [BPC, Lq, Lk], F32, kind="ExternalOutput").ap()

    with tile.TileContext(nc) as tc:
        with (
            tc.tile_pool(name="static", bufs=1) as st,
            tc.tile_pool(name="ctx", bufs=1) as ctx_pool,
            tc.tile_pool(name="qry", bufs=2) as qry_pool,
            tc.tile_pool(name="qp", bufs=2) as qp_pool,
            tc.tile_pool(name="q16s", bufs=2) as q16s_pool,
            tc.tile_pool(name="ew", bufs=2) as ew_pool,
            tc.tile_pool(name="wt", bufs=2) as wt_pool,
            tc.tile_pool(name="sm", bufs=2) as sm_pool,
            tc.tile_pool(name="ot", bufs=3) as ot_pool,
            tc.tile_pool(name="stats", bufs=4) as stats_pool,
            tc.tile_pool(name="psS", bufs=2, space="PSUM") as psS,
            tc.tile_pool(name="psO", bufs=2, space="PSUM") as psO,
            tc.tile_pool(name="psP", bufs=2, space="PSUM") as psP,
            tc.tile_pool(name="psT", bufs=2, space="PSUM") as psT,
        ):

            def qry16_dma(s, blk_i, pool=None, tag="qry"):
                q0 = blk_i * MQ
                pool = pool or qry_pool
                t = pool.tile([128, 8 * MQ], FP16, tag=tag)
                nc.sync.dma_start(
                    t[:].rearrange("p (c x) -> p c x", c=8),
                    qT16_d[s].rearrange("(c p) q -> p c q", p=128)[:, :, q0:q0 + MQ])
                return t

            def load_cT(s):
                m_, NK, prof, use_ctx, use_qp = params[s]
                S = 128 * NK
                cT_sb = ctx_pool.tile([128, 8 * Lk], FP16, tag="cT")
                nc.sync.dma_start(
                    cT_sb[:].rearrange("p (c x) -> p c x", c=8)[:, :, :S],
                    cT_d[s].rearrange("(c p) k -> p c k", p=128)[:, :, :S])
                return dict(cT=cT_sb)

            def load_masks(s, ctx):
                m_, NK, prof, use_ctx, use_qp = params[s]
                S = 128 * NK
                kmin_sb = ctx_pool.tile([128, Lk], F32, tag="kmin")
                nc.sync.dma_start(kmin_sb[:, :S], kmin_d[s, :, :S])
                qmin_sb = ctx_pool.tile([128, Lq // 128], F32, tag="qmin")
                nc.sync.dma_start(qmin_sb[:], qmin_d[s])
                q01_sb = ctx_pool.tile([128, Lq // 128], F32, tag="q01")
                nc.sync.dma_start(q01_sb[:], q01_d[s])
                ctx.update(kmin=kmin_sb, qmin=qmin_sb, q01=q01_sb)

            def cw_build(s, ctx):
                """cw[d, k] = sum_e W_in[e, d] * contextT[e, k] (fp16)."""
                m_, NK, prof, use_ctx, use_qp = params[s]
                cw_sb = ctx_pool.tile([128, 8 * Lk], FP16, tag="cw")
                chs = _chunks(128 * NK)
                for dt in range(8):
                    off = 0
                    for w in chs:
                        ps = psP.tile([128, 512], F32, tag="psP")
                        for et in range(8):
                            nc.tensor.matmul(
                                ps[:, :w],
                                win_sb[:, et * D + dt * 128:et * D + (dt + 1) * 128],
                                ctx["cT"][:, et * Lk + off:et * Lk + off + w],
                                start=(et == 0), stop=(et == 7))
                        nc.vector.tensor_copy(
                            cw_sb[:, dt * Lk + off:dt * Lk + off + w],
                            ps[:, :w])
                        off += w
                ctx["cw"] = cw_sb

            def ctxwo1_build(s, ctx):
                """kd[k, d'] = sum_d context[k, d] * Wo1[d, d'] (fp16),
                k-tile kt in columns kt*D..(kt+1)*D. Emitted late (after the
                previous slot's last out) because the kd tag frees there."""
                m_, NK, prof, use_ctx, use_qp = params[s]
                kd_sb = ctx_pool.tile([128, 8 * D], FP16, tag="kd")
                for kt in range(NK):
                    for n in range(2):
                        ps = psP.tile([128, 512], F32, tag="psP")
                        for dt in range(8):
                            nc.tensor.matmul(
                                ps[:],
                                ctx["cT"][:, dt * Lk + kt * 128:dt * Lk + (kt + 1) * 128],
                                wo1_sb[:, dt * D + n * 512:dt * D + (n + 1) * 512],
                                start=(dt == 0), stop=(dt == 7))
                        nc.vector.tensor_copy(
                            kd_sb[:, kt * D + n * 512:kt * D + (n + 1) * 512],
                            ps[:])
                ctx["kd"] = kd_sb

            def load_ctx_late(s, ctx):
                m_, NK, prof, use_ctx, use_qp = params[s]
                if use_ctx:
                    ctxwo1_build(s, ctx)
                else:
                    kd_sb = ctx_pool.tile([128, 8 * D], FP16, tag="kd")
                    nc.sync.dma_start(
                        kd_sb[:].rearrange("p (c d) -> p c d", c=8)[:, :NK],
                        cn_d[s].rearrange("(c p) d -> p c d", p=128)[:, :NK])
                    ctx["kd"] = kd_sb
                    mean_sb = ctx_pool.tile([1, D], FP16, tag="mean")
                    nc.sync.dma_start(mean_sb[:], mean_d[s:s + 1, :])
                    ctx["mean"] = mean_sb
                m01_sb = ctx_pool.tile([1, Lq], FP16, tag="m01")
                nc.sync.dma_start(m01_sb[:], m01_d[s:s + 1, :])
                cb_sb = ctx_pool.tile([1, D], FP16, tag="cb")
                nc.sync.dma_start(cb_sb[:], cb_d[s:s + 1, :])
                ctx.update(m01=m01_sb, cb=cb_sb)

            def qproj(s, blk_i, q16_sb):
                """qp[e, q] = sum_d W_in[e, d] * queryT[d, q] (fp16)."""
                qp_sb = qp_pool.tile([128, 8 * MQ], FP16, tag="qp")
                for et in range(8):
                    ps = psP.tile([128, 512], F32, tag="psP")
                    for dt in range(8):
                        nc.tensor.matmul(
                            ps[:, :MQ],
                            winT_sb[:, dt * D + et * 128:dt * D + (et + 1) * 128],
                            q16_sb[:, dt * MQ:(dt + 1) * MQ],
                            start=(dt == 0), stop=(dt == 7))
                    nc.vector.tensor_copy(qp_sb[:, et * MQ:(et + 1) * MQ],
                                          ps[:, :MQ])
                return qp_sb

            const_sb = st.tile([128, Lk], F32, tag="const")
            nc.vector.memset(const_sb[:], NEG)
            ones_sb = st.tile([1, 128], FP16, tag="ones")
            nc.vector.memset(ones_sb[:], 1.0)

            def scores_softmax(s, blk_i, qsrc_sb, ctx):
                """Masked scores -> DRAM; softmax weights -> ew tile (fp16)."""
                m_, NK, prof, use_ctx, use_qp = params[s]
                nk = prof[blk_i]
                S = 128 * nk
                chs = _chunks(S)
                NCH = len(chs)
                rhs = ctx["cT"] if use_qp else ctx["cw"]
                q0 = blk_i * MQ
                ew_sb = ew_pool.tile([128, 2 * Lk], FP16, tag="ew")
                for h in range(2):
                    jt = blk_i * 2 + h
                    rows = slice(q0 + h * 128, q0 + (h + 1) * 128)
                    stt = stats_pool.tile([128, 8], F32, tag="stats")
                    ps_n = [psS.tile([128, 512], F32, tag="psS", name="ps_n") for _ in chs]
                    for dt in range(8):
                        off = 0
                        for n, w in enumerate(chs):
                            nc.tensor.matmul(
                                ps_n[n][:, :w],
                                qsrc_sb[:, dt * MQ + h * 128:dt * MQ + (h + 1) * 128],
                                rhs[:, dt * Lk + off:dt * Lk + off + w],
                                start=(dt == 0), stop=(dt == 7))
                            off += w
                    sm_n = []
                    off = 0
                    for n, w in enumerate(chs):
                        sm = sm_pool.tile([128, 512], F32, tag="sm")
                        sm_n.append((sm, off, w))
                        nc.vector.tensor_tensor(
                            sm[:, :w], ps_n[n][:, :w], ctx["kmin"][:, off:off + w],
                            op=mybir.AluOpType.min)
                        nc.vector.tensor_scalar_min(
                            sm[:, :w], sm[:, :w], ctx["qmin"][:, jt:jt + 1])
                        nc.sync.dma_start(sc_d[s, rows, off:off + w], sm[:, :w])
                        nc.vector.reduce_max(
                            stt[:, n:n + 1], sm[:, :w],
                            axis=mybir.AxisListType.X, negate=True)
                        off += w
                    if S < Lk:
                        nc.sync.dma_start(sc_d[s, rows, S:], const_sb[:, :Lk - S])
                    if NCH == 1:
                        negm = stt[:, 0:1]
                    else:
                        nc.vector.tensor_tensor(
                            stt[:, 2:3], stt[:, 0:1], stt[:, 1:2],
                            op=mybir.AluOpType.min)
                        negm = stt[:, 2:3]
                    for n, (sm, off, w) in enumerate(sm_n):
                        nc.scalar.activation(
                            ew_sb[:, h * Lk + off:h * Lk + off + w],
                            sm[:, :w],
                            mybir.ActivationFunctionType.Exp,
                            bias=negm, scale=1.0,
                            accum_out=stt[:, 3 + n:4 + n])
                    if NCH == 1:
                        ssum = stt[:, 3:4]
                    else:
                        nc.vector.tensor_tensor(
                            stt[:, 5:6], stt[:, 3:4], stt[:, 4:5],
                            op=mybir.AluOpType.add)
                        ssum = stt[:, 5:6]
                    nc.vector.reciprocal(stt[:, 6:7], ssum)
                    if nk == 8:
                        scale = stt[:, 6:7]
                    else:
                        nc.vector.tensor_tensor(
                            stt[:, 7:8], stt[:, 6:7], ctx["q01"][:, jt:jt + 1],
                            op=mybir.AluOpType.mult)
                        scale = stt[:, 7:8]
                    nc.vector.tensor_scalar_mul(
                        ew_sb[:, h * Lk:h * Lk + S],
                        ew_sb[:, h * Lk:h * Lk + S],
                        scale)
                return ew_sb

            def transposes(s, blk_i, ew_sb):
                """W^T via PE transpose-mode (pipelines ~190ns/tile)."""
                m_, NK, prof, use_ctx, use_qp = params[s]
                nk = prof[blk_i]
                wt_sb = wt_pool.tile([128, 8 * MQ], FP16, tag="wt")
                for kt in range(nk):
                    pt = psT.tile([128, MQ], FP16, tag="psT")
                    for h in range(2):
                        nc.tensor.transpose(
                            pt[:, h * 128:(h + 1) * 128],
                            ew_sb[:, h * Lk + kt * 128:h * Lk + (kt + 1) * 128],
                            ident[:])
                    nc.vector.tensor_copy(wt_sb[:, kt * MQ:(kt + 1) * MQ], pt[:])
                return wt_sb

            def out_ctx(s, blk_i, q16_sb, wt_sb, ctx):
                """out = tanh(W~@ctxWo1 + q@Wf [+ rank1 cb]) in one psum."""
                m_, NK, prof, use_ctx, use_qp = params[s]
                nk = prof[blk_i]
                q0 = blk_i * MQ
                for h in range(2):
                    rows = slice(q0 + h * 128, q0 + (h + 1) * 128)
                    po = [psO.tile([128, 512], F32, tag="psO", name="po") for _ in range(2)]
                    for kt in range(nk):
                        for n in range(2):
                            nc.tensor.matmul(
                                po[n][:],
                                wt_sb[:, kt * MQ + h * 128:kt * MQ + (h + 1) * 128],
                                ctx["kd"][:, kt * D + n * 512:kt * D + (n + 1) * 512],
                                start=(kt == 0), stop=False)
                    for dt in range(8):
                        for n in range(2):
                            nc.tensor.matmul(
                                po[n][:],
                                q16_sb[:, dt * MQ + h * 128:dt * MQ + (h + 1) * 128],
                                wf_sb[:, dt * D + n * 512:dt * D + (n + 1) * 512],
                                start=False, stop=(dt == 7 and nk == 8))
                    if nk < 8:
                        for n in range(2):
                            nc.tensor.matmul(
                                po[n][:],
                                ctx["m01"][0:1, q0 + h * 128:q0 + (h + 1) * 128],
                                ctx["cb"][0:1, n * 512:(n + 1) * 512],
                                start=False, stop=True)
                    for n in range(2):
                        ot = ot_pool.tile([128, 512], F32, tag="ot")
                        nc.scalar.activation(
                            ot[:], po[n][:], mybir.ActivationFunctionType.Tanh)
                        nc.sync.dma_start(out_d[s, rows, n * 512:(n + 1) * 512],
                                          ot[:])

            def out_mix(s, blk_i, q16_sb, wt_sb, ctx):
                """Baseline path: mixT = kd-chunks @ wt, out via Wo1 + Wf."""
                m_, NK, prof, use_ctx, use_qp = params[s]
                nk = prof[blk_i]
                q0 = blk_i * MQ
                mixT_sb = wt_pool.tile([128, 8 * MQ], FP16, tag="mixT")
                for dt in range(8):
                    pm = psO.tile([128, 512], F32, tag="psO")
                    for kt in range(nk):
                        nc.tensor.matmul(
                            pm[:, :MQ],
                            ctx["kd"][:, kt * D + dt * 128:kt * D + (dt + 1) * 128],
                            wt_sb[:, kt * MQ:(kt + 1) * MQ],
                            start=(kt == 0), stop=(kt == nk - 1 and nk == 8))
                    if nk < 8:
                        nc.tensor.matmul(
                            pm[:, :MQ],
                            ctx["mean"][0:1, dt * 128:(dt + 1) * 128],
                            ctx["m01"][0:1, q0:q0 + MQ],
                            start=False, stop=True)
                    nc.vector.tensor_copy(mixT_sb[:, dt * MQ:(dt + 1) * MQ],
                                          pm[:, :MQ])
                for h in range(2):
                    rows = slice(q0 + h * 128, q0 + (h + 1) * 128)
                    po = [psO.tile([128, 512], F32, tag="psO", name="po") for _ in range(2)]
                    for dt in range(8):
                        for n in range(2):
                            nc.tensor.matmul(
                                po[n][:],
                                mixT_sb[:, dt * MQ + h * 128:dt * MQ + (h + 1) * 128],
                                wo1_sb[:, dt * D + n * 512:dt * D + (n + 1) * 512],
                                start=(dt == 0), stop=False)
                    for dt in range(8):
                        for n in range(2):
                            nc.tensor.matmul(
                                po[n][:],
                                q16_sb[:, dt * MQ + h * 128:dt * MQ + (h + 1) * 128],
                                wf_sb[:, dt * D + n * 512:dt * D + (n + 1) * 512],
                                start=False, stop=(dt == 7))
                    for n in range(2):
                        ot = ot_pool.tile([128, 512], F32, tag="ot")
                        nc.scalar.activation(
                            ot[:], po[n][:], mybir.ActivationFunctionType.Tanh)
                        nc.sync.dma_start(out_d[s, rows, n * 512:(n + 1) * 512],
                                          ot[:])

            def skipped_block(s, blk_i, ctx):
                """q-block past every query length in the slot: scores are
                all -1e9; out = tanh(query@Wf + cb)."""
                q0 = blk_i * MQ
                q16_sb = qry16_dma(s, blk_i, pool=q16s_pool, tag="q16s")
                for h in range(2):
                    rows = slice(q0 + h * 128, q0 + (h + 1) * 128)
                    nc.sync.dma_start(sc_d[s, rows, :], const_sb[:])
                    po = [psO.tile([128, 512], F32, tag="psO", name="po") for _ in range(2)]
                    for dt in range(8):
                        for n in range(2):
                            nc.tensor.matmul(
                                po[n][:],
                                q16_sb[:, dt * MQ + h * 128:dt * MQ + (h + 1) * 128],
                                wf_sb[:, dt * D + n * 512:dt * D + (n + 1) * 512],
                                start=(dt == 0), stop=False)
                    for n in range(2):
                        nc.tensor.matmul(
                            po[n][:], ones_sb[0:1, :],
                            ctx["cb"][0:1, n * 512:(n + 1) * 512],
                            start=False, stop=True)
                    for n in range(2):
                        ot = ot_pool.tile([128, 512], F32, tag="ot")
                        nc.scalar.activation(
                            ot[:], po[n][:], mybir.ActivationFunctionType.Tanh)
                        nc.sync.dma_start(out_d[s, rows, n * 512:(n + 1) * 512],
                                          ot[:])

            # ---- prologue
            ctx0 = load_cT(0)
            if any_cw:
                win_sb = st.tile([128, 8 * D], FP16, tag="win")
                nc.sync.dma_start(
                    win_sb[:].rearrange("p (e d) -> p e d", e=8),
                    win_d.rearrange("(e p) d -> p e d", p=128))
            if any_qp:
                winT_sb = st.tile([128, 8 * D], FP16, tag="winT")
                nc.sync.dma_start(
                    winT_sb[:].rearrange("p (c d) -> p c d", c=8),
                    winT_d.rearrange("(c p) e -> p c e", p=128))
            qry0 = qry16_dma(0, 0)
            load_masks(0, ctx0)

            if any_ctx or any_mix:
                wo1_sb = st.tile([128, 8 * D], FP16, tag="wo1")
                nc.sync.dma_start(
                    wo1_sb[:].rearrange("p (c d) -> p c d", c=8),
                    wo1_d.rearrange("(c p) d -> p c d", p=128))
            wf_sb = st.tile([128, 8 * D], FP16, tag="wf")
            nc.sync.dma_start(
                wf_sb[:].rearrange("p (c d) -> p c d", c=8),
                wf_d.rearrange("(c p) d -> p c d", p=128))
            ident = st.tile([128, 128], FP16, tag="ident")
            make_identity(nc, ident[:])

            def emit_skips(s, ctx):
                for si in range(params[s][0], NBLK):
                    skipped_block(s, si, ctx)

            if not params[0][4]:
                cw_build(0, ctx0)
            load_ctx_late(0, ctx0)

            # flattened computed-block sequence with one-block lookahead
            seq = [(s, j) for s in range(BPC) for j in range(params[s][0])]
            cur_ctx = {0: ctx0}

            def make_scores(s, j, ctx):
                q16 = qry16_dma(s, j)
                if params[s][4]:
                    qsrc = qproj(s, j, q16)
                else:
                    qsrc = q16
                ew = scores_softmax(s, j, qsrc, ctx)
                return (s, j, q16, ew)

            pend = make_scores(0, 0, ctx0)
            for idx in range(len(seq)):
                s, j = seq[idx]
                _, _, q16_sb, ew_sb = pend
                ctx = cur_ctx[s]
                if not params[s][4]:
                    for la in (2, 3):
                        if idx + la < len(seq):
                            fs = seq[idx + la][0]
                            if fs != s and fs not in cur_ctx:
                                cur_ctx[fs] = load_cT(fs)
                nxt = seq[idx + 1] if idx + 1 < len(seq) else None
                if nxt is not None:
                    ns, nj = nxt
                    if ns != s:
                        if ns not in cur_ctx:
                            cur_ctx[ns] = load_cT(ns)
                        load_masks(ns, cur_ctx[ns])
                        if not params[ns][4]:
                            cw_build(ns, cur_ctx[ns])
                    pend = make_scores(ns, nj, cur_ctx[ns])
                wt_sb = transposes(s, j, ew_sb)
                if params[s][3]:
                    out_ctx(s, j, q16_sb, wt_sb, ctx)
                else:
                    out_mix(s, j, q16_sb, wt_sb, ctx)
                if nxt is None or nxt[0] != s:
                    emit_skips(s, ctx)
                if nxt is not None and nxt[0] != s:
                    load_ctx_late(nxt[0], cur_ctx[nxt[0]])

    nc.compile()
    return nc


def kernel(query, context, query_lengths, context_lengths, W_in, W_out):
    groups, params = _assign_slots(np.asarray(query_lengths),
                                   np.asarray(context_lengths))
    if _cache.get("params") != params:
        _cache["nc"] = _build_program(params)
        _cache["params"] = params
    nc = _cache["nc"]

    query = np.asarray(query, dtype=np.float32)
    context = np.asarray(context, dtype=np.float32)
    ql = np.asarray(query_lengths).astype(np.int64)
    cl = np.asarray(context_lengths).astype(np.int64)

    qT16 = np.ascontiguousarray(query.transpose(0, 2, 1)).astype(np.float16)
    cT16 = np.ascontiguousarray(context.transpose(0, 2, 1)).astype(np.float16)
    cn16 = context.astype(np.float16)
    win = np.ascontiguousarray(W_in, dtype=np.float32).astype(np.float16)
    winT = np.ascontiguousarray(W_in.T, dtype=np.float32).astype(np.float16)
    woT = np.ascontiguousarray(W_out.T, dtype=np.float64)
    wo1 = woT[:D].astype(np.float16)
    wf = (np.asarray(W_in, dtype=np.float64).T @ woT[D:]).astype(np.float16)
    mean_c = context.astype(np.float64).mean(axis=1)
    cb = (mean_c @ woT[:D]).astype(np.float16)
    mean16 = mean_c.astype(np.float16)

    k_idx = np.arange(Lk)
    q_idx = np.arange(Lq)
    kvalid = k_idx[None, :] < cl[:, None]
    qvalid = q_idx[None, :] < ql[:, None]
    kmin = np.where(kvalid, np.float32(BIG), np.float32(NEG)).astype(np.float32)
    qmin = np.where(qvalid, np.float32(BIG), np.float32(NEG)).astype(np.float32)
    q01 = qvalid.astype(np.float32)
    m01 = (~qvalid).astype(np.float16)
    kmin_rep = np.ascontiguousarray(
        np.broadcast_to(kmin[:, None, :], (B, 128, Lk)))
    qmin_til = np.ascontiguousarray(
        qmin.reshape(B, Lq // 128, 128).transpose(0, 2, 1))
    q01_til = np.ascontiguousarray(
        q01.reshape(B, Lq // 128, 128).transpose(0, 2, 1))

    # core c processes batch groups[s][c] in slot s
    in_maps = []
    for c in range(N_CORES):
        sidx = [groups[s][c] for s in range(BPC)]
        in_maps.append({
            "qT16": np.ascontiguousarray(qT16[sidx]),
            "cT": np.ascontiguousarray(cT16[sidx]),
            "cn": np.ascontiguousarray(cn16[sidx]),
            "win": win, "winT": winT, "wo1": wo1, "wf": wf,
            "kmin": np.ascontiguousarray(kmin_rep[sidx]),
            "qmin": np.ascontiguousarray(qmin_til[sidx]),
            "q01": np.ascontiguousarray(q01_til[sidx]),
            "m01": np.ascontiguousarray(m01[sidx]),
            "mean": np.ascontiguousarray(mean16[sidx]),
            "cb": np.ascontiguousarray(cb[sidx]),
        })

    res = bass_utils.run_bass_kernel_spmd(nc, in_maps, core_ids=list(range(N_CORES)))
    _cache["last_result"] = res

    out = np.empty((B, Lq, D), dtype=np.float32)
    scores = np.empty((B, Lq, Lk), dtype=np.float32)
    for c in range(N_CORES):
        for s in range(BPC):
            bidx = groups[s][c]
            out[bidx] = res.results[c]["out"][s]
            scores[bidx] = res.results[c]["sc"][s]
    return out, scores


_program_cache = _cache  # test.py compatibility
